# revision 12
# baseline (speedup 1.0000x reference)
"""Multi-head attention (B=2, T=2048, C=1024, H=16, D=64) on 8 TRN2 cores.

Tensor-parallel over heads: each core owns 2 heads (128 channels).
Per core:
  - q,k projected channel-major (qT/kT [128, N]); bk dropped (softmax
    shift-invariance), bq added at PSUM evict.
  - v projected token-major [N, 128]; bv folded into host-side bias
    (softmax rows sum to 1 => bv passes through attention unchanged).
  - attention with scoresT = k @ q.T layout ([ktok, qtok]); exp on ScalarE
    with the 1/sqrt(D) scale folded in; no max-subtraction (|scores| < ~4).
  - v augmented with a ones column (lhsT M=65) so the PV matmul also
    accumulates softmax denominators in PSUM row 64.
  - normalize at PV evict: reciprocal + DMA partition-broadcast + mul.
  - partial o_proj with Wo column slice; host sums the 8 partials + bias.
"""

import contextlib

import numpy as np

import concourse.bacc as bacc
import concourse.bass as bass
import concourse.tile as tile
from concourse import mybir
from concourse.bass_utils import run_bass_kernel_spmd

NCORES = 8
B, T, C, H, D = 2, 2048, 1024, 16, 64
N = B * T  # 4096 tokens
CPC = 128  # channels per core (2 heads x 64)
SCALE = 0.125  # 1/sqrt(64)
F32 = mybir.dt.float32

KT = C // 128  # 8 contraction tiles for projections
NCH = N // 512  # 8 token chunks for projections
TTOK = N // 128  # 32 token tiles
KTA = T // 128  # 16 k tiles per batch in attention
QC = T // 512  # 4 q chunks per batch
VS = 2 * (D + 1)  # 130: per-k-tile stride in v_aug (65 cols per head)
KG = 3  # k-tiles per exp batch (3 PSUM banks per head)

_CACHE = {}


def _free_ap(ap, dims):
    """AP with the same tensor/offset/partition dim but custom free dims."""
    return bass.AP(tensor=ap.tensor, offset=ap.offset, ap=[ap.ap[0]] + dims)


def _build(dbg=False):
    nc = bacc.Bacc("TRN2", target_bir_lowering=False, debug=False)

    xT = nc.dram_tensor("xT", [C, N], F32, kind="ExternalInput")
    wqT = nc.dram_tensor("wqT", [C, CPC], F32, kind="ExternalInput")
    wkT = nc.dram_tensor("wkT", [C, CPC], F32, kind="ExternalInput")
    wvT = nc.dram_tensor("wvT", [C, CPC], F32, kind="ExternalInput")
    woT = nc.dram_tensor("woT", [CPC, C], F32, kind="ExternalInput")
    bq = nc.dram_tensor("bq", [CPC, 1], F32, kind="ExternalInput")
    out = nc.dram_tensor("out", [N, C], F32, kind="ExternalOutput")
    if dbg:
        d_qT = nc.dram_tensor("d_qT", [128, N], F32, kind="ExternalOutput")
        d_kT = nc.dram_tensor("d_kT", [128, N], F32, kind="ExternalOutput")
        d_va = nc.dram_tensor("d_va", [128, TTOK * VS], F32, kind="ExternalOutput")
        d_ao = nc.dram_tensor("d_ao", [128, N], F32, kind="ExternalOutput")

    with tile.TileContext(nc) as tc, tc.tile_pool(name="persist", bufs=1) as persist:
        qT_sb = persist.tile([128, N], F32, tag="qT")
        kT_sb = persist.tile([128, N], F32, tag="kT")
        vaug = persist.tile([128, TTOK * VS], F32, tag="vaug")
        aout = persist.tile([128, N], F32, tag="aout")
        wq_sb = persist.tile([128, C], F32, tag="wq")
        wk_sb = persist.tile([128, C], F32, tag="wk")
        wv_sb = persist.tile([128, C], F32, tag="wv")
        wo_sb = persist.tile([128, C], F32, tag="wo")
        bq_sb = persist.tile([128, 1], F32, tag="bq")

        for kt in range(KT):
            ksl = slice(kt * 128, (kt + 1) * 128)
            nc.sync.dma_start(out=wq_sb[:, ksl], in_=wqT[ksl, :])
            nc.sync.dma_start(out=wk_sb[:, ksl], in_=wkT[ksl, :])
            nc.sync.dma_start(out=wv_sb[:, ksl], in_=wvT[ksl, :])
        nc.sync.dma_start(out=wo_sb[:, :], in_=woT[:, :])
        nc.sync.dma_start(out=bq_sb[:, :], in_=bq[:, :])
        nc.vector.memset(vaug[:, :], 1.0)

        # ---- Phase A: projections -------------------------------------
        with (
            tc.tile_pool(name="xk", bufs=12) as xkp,
            tc.tile_pool(name="psA", bufs=2, space="PSUM") as psA,
            tc.tile_pool(name="psV", bufs=4, space="PSUM") as psV,
        ):
            for nch in range(NCH):
                cols = slice(nch * 512, (nch + 1) * 512)
                xks = []
                for kt in range(KT):
                    xk = xkp.tile([128, 512], F32, tag="xk")
                    nc.sync.dma_start(
                        out=xk[:, :], in_=xT[kt * 128 : (kt + 1) * 128, cols]
                    )
                    xks.append(xk)
                psq = psA.tile([128, 512], F32, tag="psq")
                psk = psA.tile([128, 512], F32, tag="psk")
                psv = [
                    psV.tile([128, 128], F32, tag="psv", name=f"psv{tt}")
                    for tt in range(4)
                ]
                for kt in range(KT):
                    ksl = slice(kt * 128, (kt + 1) * 128)
                    st, sp = kt == 0, kt == KT - 1
                    nc.tensor.matmul(
                        psq[:, :], lhsT=wq_sb[:, ksl], rhs=xks[kt][:, :],
                        start=st, stop=sp,
                    )
                    nc.tensor.matmul(
                        psk[:, :], lhsT=wk_sb[:, ksl], rhs=xks[kt][:, :],
                        start=st, stop=sp,
                    )
                    for tt in range(4):
                        nc.tensor.matmul(
                            psv[tt][:, :],
                            lhsT=xks[kt][:, tt * 128 : (tt + 1) * 128],
                            rhs=wv_sb[:, ksl],
                            start=st, stop=sp,
                        )
                nc.vector.tensor_scalar_add(
                    out=qT_sb[:, cols], in0=psq[:, :], scalar1=bq_sb[:, :]
                )
                nc.vector.tensor_copy(out=kT_sb[:, cols], in_=psk[:, :])
                for tt in range(4):
                    g = nch * 4 + tt  # global token tile
                    for h in range(2):
                        nc.vector.tensor_copy(
                            out=vaug[:, g * VS + h * 65 : g * VS + h * 65 + 64],
                            in_=psv[tt][:, h * 64 : h * 64 + 64],
                        )

        # ---- Phase B: attention ---------------------------------------
        with (
            tc.tile_pool(name="psS", bufs=1, space="PSUM") as psS,
            tc.tile_pool(name="psP", bufs=1, space="PSUM") as psP,
            tc.tile_pool(name="aup", bufs=2) as aup,
            tc.tile_pool(name="nrm", bufs=4) as nrm,
            tc.tile_pool(name="drs", bufs=4, space="DRAM") as drs,
        ):
            for b in range(B):
                for qc in range(QC):
                    q0 = b * T + qc * 512
                    qsl = slice(q0, q0 + 512)
                    pv = [
                        psP.tile([65, 512], F32, tag=f"pv{h}", name=f"pv{h}")
                        for h in range(2)
                    ]
                    for kg in range((KTA + KG - 1) // KG):
                        kts = list(range(kg * KG, min((kg + 1) * KG, KTA)))
                        ng = len(kts)
                        sc = psS.tile([128, 2 * KG * 512], F32, tag="sc")
                        for i, kt in enumerate(kts):
                            kcols = slice(b * T + kt * 128, b * T + (kt + 1) * 128)
                            for h in range(2):
                                hp = slice(h * 64, (h + 1) * 64)
                                nc.tensor.matmul(
                                    sc[:, (h * KG + i) * 512 : (h * KG + i + 1) * 512],
                                    lhsT=kT_sb[hp, kcols],
                                    rhs=qT_sb[hp, qsl],
                                    start=True, stop=True,
                                )
                        au = aup.tile([128, 2 * KG * 512], F32, tag="au")
                        nc.scalar.activation(
                            out=_free_ap(au[:, :], [[ng * 512, 2], [1, ng * 512]]),
                            in_=_free_ap(sc[:, :], [[KG * 512, 2], [1, ng * 512]]),
                            func=mybir.ActivationFunctionType.Exp,
                            scale=SCALE,
                        )
                        for i, kt in enumerate(kts):
                            g = b * KTA + kt
                            for h in range(2):
                                nc.tensor.matmul(
                                    pv[h][:, :],
                                    lhsT=vaug[:, g * VS + h * 65 : g * VS + (h + 1) * 65],
                                    rhs=au[:, (h * ng + i) * 512 : (h * ng + i + 1) * 512],
                                    start=(kt == 0), stop=(kt == KTA - 1),
                                )
                    for h in range(2):
                        rc = nrm.tile([1, 512], F32, tag="rc")
                        nc.vector.reciprocal(out=rc[:, :], in_=pv[h][64:65, :])
                        rcd = drs.tile([1, 512], F32, tag="rcd")
                        nc.sync.dma_start(out=rcd[:, :], in_=rc[:, :])
                        rc64 = nrm.tile([64, 512], F32, tag="rc64")
                        dap = rcd[:, :]
                        nc.sync.dma_start(
                            out=rc64[:, :],
                            in_=bass.AP(
                                tensor=dap.tensor, offset=dap.offset,
                                ap=[[0, 64]] + list(dap.ap[1:]),
                            ),
                        )
                        nc.vector.tensor_mul(
                            out=aout[h * 64 : (h + 1) * 64, qsl],
                            in0=pv[h][0:64, :],
                            in1=rc64[:, :],
                        )

        if dbg:
            nc.sync.dma_start(out=d_qT[:, :], in_=qT_sb[:, :])
            nc.sync.dma_start(out=d_kT[:, :], in_=kT_sb[:, :])
            nc.sync.dma_start(out=d_va[:, :], in_=vaug[:, :])
            nc.sync.dma_start(out=d_ao[:, :], in_=aout[:, :])

        # ---- Phase C: partial o_proj ----------------------------------
        with (
            tc.tile_pool(name="psC", bufs=2, space="PSUM") as psC,
            tc.tile_pool(name="ob", bufs=3) as obp,
        ):
            for tt in range(TTOK):
                po = psC.tile([128, 1024], F32, tag="po")
                for nh in range(2):
                    nc.tensor.matmul(
                        po[:, nh * 512 : (nh + 1) * 512],
                        lhsT=aout[:, tt * 128 : (tt + 1) * 128],
                        rhs=wo_sb[:, nh * 512 : (nh + 1) * 512],
                        start=True, stop=True,
                    )
                ob = obp.tile([128, 1024], F32, tag="ob")
                nc.vector.tensor_copy(out=ob[:, :], in_=po[:, :])
                nc.sync.dma_start(
                    out=out[tt * 128 : (tt + 1) * 128, :], in_=ob[:, :]
                )

    nc.compile()
    return nc


def _prep_inputs(x_q, Wq, bq, Wk, Wv, Wo):
    x = np.ascontiguousarray(np.asarray(x_q, np.float32).reshape(N, C))
    xT = np.ascontiguousarray(x.T)
    Wq = np.asarray(Wq, np.float32)
    Wk = np.asarray(Wk, np.float32)
    Wv = np.asarray(Wv, np.float32)
    Wo = np.asarray(Wo, np.float32)
    bq = np.asarray(bq, np.float32)
    in_maps = []
    for c in range(NCORES):
        sl = slice(c * CPC, (c + 1) * CPC)
        in_maps.append(
            {
                "xT": xT,
                "wqT": np.ascontiguousarray(Wq[sl, :].T),
                "wkT": np.ascontiguousarray(Wk[sl, :].T),
                "wvT": np.ascontiguousarray(Wv[sl, :].T),
                "woT": np.ascontiguousarray(Wo[:, sl].T),
                "bq": np.ascontiguousarray(bq[sl].reshape(CPC, 1)),
            }
        )
    return in_maps


def _finish(results, Wo, bv, bo):
    acc = results[0]["out"].astype(np.float32)
    for r in results[1:]:
        acc = acc + r["out"]
    bo_eff = np.asarray(bo, np.float32) + np.asarray(Wo, np.float32) @ np.asarray(
        bv, np.float32
    )
    return (acc + bo_eff[None, :]).reshape(B, T, C).astype(np.float32)


def run(inputs, trace=False, **kw):
    if "nc" not in _CACHE:
        _CACHE["nc"] = _build()
    nc = _CACHE["nc"]
    in_maps = _prep_inputs(
        inputs["x_q"], inputs["Wq"], inputs["bq"], inputs["Wk"], inputs["Wv"],
        inputs["Wo"],
    )
    res = run_bass_kernel_spmd(nc, in_maps, core_ids=list(range(NCORES)),
                               trace=trace, **kw)
    out = _finish(res.results, inputs["Wo"], inputs["bv"], inputs["bo"])
    return out, res


def kernel(**inputs):
    out, _ = run(inputs)
    return out
